# revision 72
# baseline (speedup 1.0000x reference)
"""Multi-head self-attention (RoPE, causal) Trainium2 Bass kernel.

Sharding: 8 cores = 2 batches x 4 head-groups (4 heads each).
Each core computes QKV projections for its heads (feature-major via x^T),
RoPE, causal attention with transposed scores (softmax along partitions
handled via exp + ones-column denominator in the V matmul), and a partial
output projection over its head slice. The host sums the 4 partials per
batch (reduce step of the tensor-parallel output projection).

v2: bf16 data paths (inputs, Q/K/V, E, attT, weights, output partials;
host upcasts before the 4-partial reduce) with fp32 PSUM accumulation;
ordered input DMAs so compute starts after ~2 MB arrives;
the RoPE even/odd swap runs as a PE permutation matmul instead of
SBUF->SBUF DMAs; softmax denominator broadcast via gpsimd
partition_broadcast (no PE broadcast matmul); elementwise work spread
across ACT/DVE/Pool.

Scheduling: attention for query blocks 0-1 is emitted inside phase A on
mini PSUM rings, hiding it under the remaining QKV projections; per-head
attention passes use single-alloc score tiles so a 2-deep ring gives a
full pair of lookahead for the ACT exp stream; AV matmuls trail one pair
behind scores; Wo tiles of block j-1 are spread across block j's passes
as PE filler; the final pass normalizes per 128-col chunk and drains the
last Wo tiles progressively.

TimelineSim: 147.7 us vs 224.5 us for the fp32r baseline (measured
258.5 us on HW); HW rel err vs reference 4.8e-3.
"""
import math
from contextlib import ExitStack

import numpy as np

import concourse.tile as tile
from concourse import bacc, mybir

F32 = mybir.dt.float32
BF16 = mybir.dt.bfloat16
EXP = mybir.ActivationFunctionType.Exp

B, S, D, H, DH = 2, 2048, 1024, 16, 64
THETA = 10000.0
CORES = 8
HPC = 4                    # heads per core
F = HPC * DH               # 256 features per core
SCALE = 1.0 / math.sqrt(DH)
NKT = D // 128             # 8 k tiles
NSB = S // 512             # 4 seq blocks of 512
NST = S // 128             # 16 seq tiles of 128

DEFAULT_OPTS = dict(reps=1, rope_add_pool=True)

_CACHED = {}


def _build_program(opts):
    reps = opts["reps"]
    nc = bacc.Bacc("TRN2", target_bir_lowering=False, debug=False,
                   num_devices=CORES)

    xT = nc.dram_tensor("xT", [D, S], BF16, kind="ExternalInput")
    wqT = nc.dram_tensor("wqT", [D, F], BF16, kind="ExternalInput")
    wkT = nc.dram_tensor("wkT", [D, F], BF16, kind="ExternalInput")
    wvT = nc.dram_tensor("wvT", [D, F], BF16, kind="ExternalInput")
    woT = nc.dram_tensor("woT", [F, D], BF16, kind="ExternalInput")
    ropeA_d = nc.dram_tensor("ropeA", [128, S], BF16, kind="ExternalInput")
    ropeB2_d = nc.dram_tensor("ropeB2", [128, S], BF16, kind="ExternalInput")
    tri_d = nc.dram_tensor("tri", [128, 128], BF16, kind="ExternalInput")
    perm_d = nc.dram_tensor("perm", [128, 128], BF16, kind="ExternalInput")

    out_d = nc.dram_tensor("partial", [S, D], BF16, kind="ExternalOutput")

    with tile.TileContext(nc) as tc, ExitStack() as ctx:
        persist = ctx.enter_context(tc.tile_pool(name="persist", bufs=1))

        # ---- persistent tiles ----
        # packed weights: wq/wk/wv are [128, 8k x 256f]; wo is [128, 2t x 1024]
        wq_t = persist.tile([128, NKT * F], BF16, tag="wq", name="wq")
        wk_t = persist.tile([128, NKT * F], BF16, tag="wk", name="wk")
        wv_t = persist.tile([128, NKT * F], BF16, tag="wv", name="wv")
        wo_t = persist.tile([128, 2 * D], BF16, tag="wo", name="wo")
        ropeA = persist.tile([128, S], BF16, tag="ropeA", name="ropeA")
        ropeB2 = persist.tile([128, S], BF16, tag="ropeB2", name="ropeB2")
        tri = persist.tile([128, 128], BF16, tag="tri", name="tri")
        perm = persist.tile([128, 128], BF16, tag="perm", name="perm")
        # x, packed feature-major: [128, 8k x 2048s], column block sb holds
        # slices [k*2048 + 512*sb : ...] per k
        xt = persist.tile([128, NKT * S], BF16, tag="xt", name="xt")
        QT = [persist.tile([128, S], BF16, tag=f"QT{t}", name=f"QT{t}") for t in range(2)]
        KT = [persist.tile([128, S], BF16, tag=f"KT{t}", name=f"KT{t}") for t in range(2)]
        Vaug = [persist.tile([128, 260], BF16, tag=f"Vaug{st}", name=f"Vaug{st}")
                for st in range(NST)]
        attT = [persist.tile([128, S], BF16, tag=f"attT{t}", name=f"attT{t}") for t in range(2)]
        for st in range(NST):
            nc.vector.memset(Vaug[st][:, 64:260:65], 1.0)

        # ---- input loads, in consumption order ----
        def _w_load(dst, w_dram):
            # [1024, 256] dram -> [128, 8x256] sbuf, one DMA
            nc.sync.dma_start(
                out=dst.rearrange("p (k c) -> p k c", k=NKT),
                in_=w_dram.rearrange("(k p) c -> p k c", p=128))

        def _w_load_half(dst, w_dram, h):
            kk = slice(NKT // 2 * h, NKT // 2 * (h + 1))
            nc.sync.dma_start(
                out=dst.rearrange("p (k c) -> p k c", k=NKT)[:, kk],
                in_=w_dram.rearrange("(k p) c -> p k c", p=128)[:, kk])

        _w_load_half(wq_t, wqT, 0)
        _w_load_half(wq_t, wqT, 1)
        for sb in range(NSB):
            cs = slice(512 * sb, 512 * (sb + 1))
            for k in range(NKT):
                nc.sync.dma_start(
                    out=xt[:, S * k + 512 * sb: S * k + 512 * (sb + 1)],
                    in_=xT[128 * k:128 * (k + 1), cs])
            if sb == 0:
                _w_load(wk_t, wkT)
                nc.sync.dma_start(out=ropeA, in_=ropeA_d[:, :])
                nc.sync.dma_start(out=ropeB2, in_=ropeB2_d[:, :])
                nc.sync.dma_start(out=perm, in_=perm_d[:, :])
                _w_load(wv_t, wvT)
        nc.sync.dma_start(
            out=wo_t.rearrange("p (t c) -> p t c", t=2),
            in_=woT.rearrange("(t p) c -> p t c", p=128))
        nc.sync.dma_start(out=tri, in_=tri_d[:, :])

        env = dict(
            wq_t=wq_t, wk_t=wk_t, wv_t=wv_t, wo_t=wo_t, ropeA=ropeA,
            ropeB2=ropeB2, tri=tri, perm=perm, xt=xt, QT=QT, KT=KT,
            Vaug=Vaug, attT=attT, out_d=out_d,
        )
        for _rep in range(reps):
            _body(nc, tc, opts, env)

    nc.compile()
    return nc


def _body(nc, tc, opts, env):
    wq_t = env["wq_t"]; wk_t = env["wk_t"]; wv_t = env["wv_t"]
    wo_t = env["wo_t"]; ropeA = env["ropeA"]; ropeB2 = env["ropeB2"]
    tri = env["tri"]; perm = env["perm"]; xt = env["xt"]
    QT = env["QT"]; KT = env["KT"]; Vaug = env["Vaug"]; attT = env["attT"]
    out_d = env["out_d"]

    def xts(k, s0, s1):
        return xt[:, S * k + s0: S * k + s1]

    with tc.tile_pool(name="ptmp", bufs=4) as ptmp, \
         tc.tile_pool(name="epool", bufs=8) as epool, \
         tc.tile_pool(name="ntmp", bufs=6) as ntmp, \
         tc.tile_pool(name="opool", bufs=6) as opool:

        def attention_block(j, score_alloc, pn_alloc, wo_queue, wo_spread,
                            emit_wo_tile, half_tiles=False, passes=None):
            """Causal attention for query block j (4 per-head passes).

            half_tiles: allocate per-key-tile [128,512] score tiles and exp
            each half separately (used by the phase-A minis: same PSUM
            footprint buys a deeper ring at the cost of extra ACT overhead,
            which phase A has slack for)."""
            qs = slice(512 * j, 512 * (j + 1))
            n_pair = 2 * (j + 1)
            for (hp, hh) in (passes if passes is not None
                             else [(0, 0), (0, 1), (1, 0), (1, 1)]):
                    t = hp
                    rs = slice(64 * hh, 64 * (hh + 1))
                    h = 2 * hp + hh
                    vc = slice(65 * (h % HPC), 65 * (h % HPC) + 65)
                    pn = pn_alloc()

                    def emit_av(p, Epair, roffs):
                        ra, rb = roffs
                        nc.tensor.matmul(pn[:, ra:512],
                                         Vaug[2 * p][:, vc],
                                         Epair[0][:, ra:512],
                                         start=(p == 0), stop=False)
                        nc.tensor.matmul(pn[:, rb:512],
                                         Vaug[2 * p + 1][:, vc],
                                         Epair[1][:, rb:512],
                                         start=False, stop=(p == n_pair - 1))

                    pend = None
                    for p in range(n_pair):
                        diag = p >= n_pair - 2
                        r0 = 256 * (p - (n_pair - 2)) if diag else 0
                        roffs = (r0, r0 + 128) if diag else (0, 0)
                        if half_tiles:
                            Epair = []
                            for (sk, r) in ((2 * p, roffs[0]),
                                            (2 * p + 1, roffs[1])):
                                ks = slice(128 * sk, 128 * (sk + 1))
                                qsr = slice(512 * j + r, 512 * (j + 1))
                                psH = score_alloc()
                                nc.tensor.matmul(psH[:, r:512],
                                                 KT[t][rs, ks], QT[t][rs, qsr],
                                                 start=True, stop=True)
                                Eh = epool.tile([128, 512], BF16, tag="Eh",
                                                name="Eh")
                                nc.scalar.activation(out=Eh[:, r:512],
                                                     in_=psH[:, r:512],
                                                     func=EXP, scale=SCALE)
                                if diag:
                                    nc.vector.tensor_mul(
                                        Eh[:, r:r + 128],
                                        Eh[:, r:r + 128], tri)
                                Epair.append(Eh)
                        else:
                            psS = score_alloc()
                            for (sk, hbase, r) in ((2 * p, 0, roffs[0]),
                                                   (2 * p + 1, 512, roffs[1])):
                                ks = slice(128 * sk, 128 * (sk + 1))
                                qsr = slice(512 * j + r, 512 * (j + 1))
                                nc.tensor.matmul(psS[:, hbase + r:hbase + 512],
                                                 KT[t][rs, ks], QT[t][rs, qsr],
                                                 start=True, stop=True)
                            E = epool.tile([128, 1024], BF16, tag="E", name="E")
                            if not diag:
                                nc.scalar.activation(out=E, in_=psS,
                                                     func=EXP, scale=SCALE)
                            else:
                                for (hbase, r) in ((0, roffs[0]),
                                                   (512, roffs[1])):
                                    nc.scalar.activation(
                                        out=E[:, hbase + r:hbase + 512],
                                        in_=psS[:, hbase + r:hbase + 512],
                                        func=EXP, scale=SCALE)
                                    nc.vector.tensor_mul(
                                        E[:, hbase + r:hbase + r + 128],
                                        E[:, hbase + r:hbase + r + 128], tri)
                            Epair = (E[:, 0:512], E[:, 512:1024])
                        if wo_queue and p == min(1, n_pair - 1):
                            for _ in range(min(wo_spread, len(wo_queue))):
                                emit_wo_tile(*wo_queue.pop(0))
                        if pend is not None:
                            emit_av(*pend)
                        pend = (p, Epair, roffs)
                    emit_av(*pend)
                    # normalize -> attT
                    rc1 = ntmp.tile([1, 512], F32, tag="rc1", name="rc1")
                    nc.vector.reciprocal(rc1, pn[64:65, :])
                    rcb = ntmp.tile([64, 512], F32, tag="rcb", name="rcb")
                    nc.gpsimd.partition_broadcast(rcb, rc1, channels=64)
                    if not (j == NSB - 1 and hp == 1 and hh == 1):
                        nc.vector.tensor_mul(attT[t][rs, qs], pn[0:64, :], rcb)
                    else:
                        # final pass: normalize per 128-col chunk and emit the
                        # last block's Wo tiles progressively (shrinks tail)
                        for st in range(4):
                            c = slice(128 * st, 128 * (st + 1))
                            qc = slice(512 * j + 128 * st,
                                       512 * j + 128 * (st + 1))
                            nc.vector.tensor_mul(attT[t][rs, qc],
                                                 pn[0:64, c], rcb[:, c])
                            emit_wo_tile(j, st, 0, ob_act=False)
                            emit_wo_tile(j, st, 1, ob_act=True)

        # ---- Phase A: QKV projections + RoPE; attention j=0,1 overlapped ----
        with tc.tile_pool(name="psProj", bufs=2, space="PSUM") as psProj, \
             tc.tile_pool(name="psSwap", bufs=1, space="PSUM") as psSwap, \
             tc.tile_pool(name="psV", bufs=2, space="PSUM") as psV, \
             tc.tile_pool(name="psM", bufs=1, space="PSUM") as psM:
            for sb in range(NSB):
                c0 = 512 * sb
                for (w_t, dest) in ((wq_t, QT), (wk_t, KT)):
                    for t in range(2):
                        ps = psProj.tile([128, 512], F32, tag="proj", name="proj")
                        for k in range(NKT):
                            lhsT = w_t[:, F * k + 128 * t: F * k + 128 * (t + 1)]
                            nc.tensor.matmul(ps, lhsT, xts(k, c0, c0 + 512),
                                             start=(k == 0), stop=(k == NKT - 1))
                        # rope: dest = raw*A + swap(raw)*B2
                        raw = ptmp.tile([128, 512], BF16, tag="raw", name="raw")
                        nc.scalar.copy(raw, ps)
                        t2p = psSwap.tile([128, 512], F32, tag="t2p", name="t2p")
                        nc.tensor.matmul(t2p, perm, raw, start=True, stop=True)
                        t1 = ptmp.tile([128, 512], BF16, tag="t1", name="t1")
                        nc.vector.tensor_mul(t1, raw, ropeA[:, c0:c0 + 512])
                        t2 = ptmp.tile([128, 512], BF16, tag="t2", name="t2")
                        nc.vector.tensor_mul(t2, t2p, ropeB2[:, c0:c0 + 512])
                        if opts["rope_add_pool"]:
                            nc.gpsimd.tensor_add(dest[t][:, c0:c0 + 512], t1, t2)
                        else:
                            nc.vector.tensor_add(dest[t][:, c0:c0 + 512], t1, t2)
                # V projection for this block (seq-major)
                for st in range(4 * sb, 4 * sb + 4):
                    s0 = 128 * st
                    ps = psV.tile([128, 256], F32, tag="projv", name="projv")
                    for k in range(NKT):
                        nc.tensor.matmul(ps, xts(k, s0, s0 + 128),
                                         wv_t[:, F * k: F * (k + 1)],
                                         start=(k == 0), stop=(k == NKT - 1))
                    dst = Vaug[st][:, 0:260].rearrange("p (h c) -> p h c", h=HPC)
                    nc.scalar.copy(dst[:, :, 0:64],
                                   ps.rearrange("p (h c) -> p h c", h=HPC))
                # early attention for blocks 0 and 1 overlaps the remaining
                # projections (mini psum rings; Wo deferred to phase B)
                if sb <= 1:
                    attention_block(
                        sb,
                        lambda: psM.tile([128, 512], F32, tag="scoreM",
                                         name="scoreM", bufs=2),
                        lambda: psM.tile([65, 512], F32, tag="pnM",
                                         name="pnM", bufs=1),
                        [], 0, None, half_tiles=True)

        # ---- Phase B: attention j=2,3 + all Wo tiles ----
        with tc.tile_pool(name="psA", bufs=2, space="PSUM") as psA, \
             tc.tile_pool(name="psB", bufs=2, space="PSUM") as psB:
            def emit_wo_tile(j, st, ot, ob_act=False):
                stg = 4 * j + st
                ss = slice(128 * stg, 128 * (stg + 1))
                os_ = slice(512 * ot, 512 * (ot + 1))
                pw = psB.tile([128, 512], F32, tag="pw", name="pw")
                for t in range(2):
                    nc.tensor.matmul(pw, attT[t][:, ss],
                                     wo_t[:, D * t + 512 * ot: D * t + 512 * (ot + 1)],
                                     start=(t == 0), stop=(t == 1))
                ob = opool.tile([128, 512], BF16, tag="ob", name="ob")
                if ob_act:
                    nc.scalar.copy(ob, pw)
                else:
                    nc.vector.tensor_copy(ob, pw)
                nc.sync.dma_start(out=out_d[ss, os_], in_=ob)

            wo_tiles = lambda j: [(j, s, o) for s in range(4) for o in range(2)]
            attention_block(
                2,
                lambda: psA.tile([128, 1024], F32, tag="score", name="score"),
                lambda: psB.tile([65, 512], F32, tag="pn", name="pn"),
                wo_tiles(0) + wo_tiles(1), 4, emit_wo_tile)
            attention_block(
                3,
                lambda: psA.tile([128, 1024], F32, tag="score", name="score"),
                lambda: psB.tile([65, 512], F32, tag="pn", name="pn"),
                wo_tiles(2), 2, emit_wo_tile)


def get_program(use_rs=False, reps=1, **kw):
    opts = dict(DEFAULT_OPTS)
    opts.update(reps=reps, **kw)
    key = tuple(sorted(opts.items()))
    if key not in _CACHED:
        _CACHED[key] = _build_program(opts)
    return _CACHED[key]


def make_in_maps(x, Wq, Wk, Wv, Wo, token_positions):
    """Host-side sharding: per-core input dicts."""
    import ml_dtypes
    bf16 = ml_dtypes.bfloat16
    x = np.asarray(x, dtype=np.float32)
    Wq = np.asarray(Wq, dtype=np.float32)
    Wk = np.asarray(Wk, dtype=np.float32)
    Wv = np.asarray(Wv, dtype=np.float32)
    Wo = np.asarray(Wo, dtype=np.float32)
    pos = np.asarray(token_positions).astype(np.float32)

    # rope tables, feature-major [128, S]: row p -> pair index i = p % 32,
    # rows [0:32]=evens, [32:64]=odds per 64-row head block.
    i = np.arange(DH // 2, dtype=np.float32)
    d = THETA ** (2.0 * i / DH)                       # [32]
    tt = pos[None, :] / d[:, None]                    # [32, S]
    sin, cos = np.sin(tt), np.cos(tt)
    A = np.tile(cos, (4, 1)).astype(bf16)             # [128, S]
    # B (applied to pre-swapped raw): evens rows -> -sin, odds rows -> +sin
    B2 = np.tile(np.concatenate([-sin, sin], axis=0), (2, 1)).astype(bf16)

    # causal triangle mask [128, 128]: allow j >= p
    p = np.arange(128)[:, None]
    jj = np.arange(128)[None, :]
    tri = (jj >= p).astype(bf16)

    # swap permutation: swapped = perm.T @ raw ; perm[i, j] = 1 where
    # out row j reads in row i = j ^ 32 (swap 32-row groups pairwise)
    pm = np.zeros((128, 128), np.float32)
    for jr in range(128):
        pm[jr ^ 32, jr] = 1.0
    pm = pm.astype(bf16)

    # per-head Q/K row permutation: evens then odds
    i2 = np.arange(DH // 2)
    perm_rows = np.concatenate(
        [np.concatenate([64 * h + 2 * i2, 64 * h + 2 * i2 + 1]) for h in range(H)])

    in_maps = []
    for c in range(CORES):
        b, g = c // 4, c % 4
        rows = perm_rows[F * g:F * (g + 1)]
        nat = np.arange(F * g, F * (g + 1))
        in_maps.append({
            "xT": np.ascontiguousarray(x[b].T).astype(bf16),
            "wqT": np.ascontiguousarray(Wq[rows, :].T).astype(bf16),
            "wkT": np.ascontiguousarray(Wk[rows, :].T).astype(bf16),
            "wvT": np.ascontiguousarray(Wv[nat, :].T).astype(bf16),
            "woT": np.ascontiguousarray(Wo[:, nat].T).astype(bf16),
            "ropeA": A,
            "ropeB2": B2,
            "tri": tri,
            "perm": pm,
        })
    return in_maps


def kernel(x, Wq, Wk, Wv, Wo, token_positions):
    from concourse.bass_utils import run_bass_kernel_spmd
    nc = get_program(False)
    in_maps = make_in_maps(x, Wq, Wk, Wv, Wo, token_positions)
    res = run_bass_kernel_spmd(nc, in_maps, list(range(CORES)))
    out = np.empty((B, S, D), dtype=np.float32)
    for b in range(B):
        acc = res.results[4 * b]["partial"].astype(np.float32).copy()
        for g in range(1, 4):
            acc += res.results[4 * b + g]["partial"]
        out[b] = acc
    return out


# revision 73
# speedup vs baseline: 1.0205x; 1.0205x over previous
"""Multi-head self-attention (RoPE, causal) Trainium2 Bass kernel.

Sharding: 8 cores = 2 batches x 4 head-groups (4 heads each).
Each core computes QKV projections for its heads (feature-major via x^T),
RoPE, causal attention with transposed scores (softmax along partitions
handled via exp + ones-column denominator in the V matmul), and a partial
output projection over its head slice. The host sums the 4 partials per
batch (reduce step of the tensor-parallel output projection).

v2: bf16 data paths (inputs, Q/K/V, E, attT, weights, output partials;
host upcasts before the 4-partial reduce) with fp32 PSUM accumulation;
ordered input DMAs so compute starts after ~2 MB arrives;
the RoPE even/odd swap runs as a PE permutation matmul instead of
SBUF->SBUF DMAs; softmax denominator broadcast via gpsimd
partition_broadcast (no PE broadcast matmul); elementwise work spread
across ACT/DVE/Pool.

Scheduling: attention for query blocks 0-1 is emitted inside phase A on
mini PSUM rings, hiding it under the remaining QKV projections; per-head
attention passes use single-alloc score tiles so a 2-deep ring gives a
full pair of lookahead for the ACT exp stream; AV matmuls trail one pair
behind scores; Wo tiles of block j-1 are spread across block j's passes
as PE filler; the final pass normalizes per 128-col chunk and drains the
last Wo tiles progressively.

TimelineSim: 147.7 us vs 224.5 us for the fp32r baseline (measured
258.5 us on HW); HW rel err vs reference 4.8e-3.
"""
import math
from contextlib import ExitStack

import numpy as np

import concourse.tile as tile
from concourse import bacc, mybir

F32 = mybir.dt.float32
BF16 = mybir.dt.bfloat16
EXP = mybir.ActivationFunctionType.Exp

B, S, D, H, DH = 2, 2048, 1024, 16, 64
THETA = 10000.0
CORES = 8
HPC = 4                    # heads per core
F = HPC * DH               # 256 features per core
SCALE = 1.0 / math.sqrt(DH)
NKT = D // 128             # 8 k tiles
NSB = S // 512             # 4 seq blocks of 512
NST = S // 128             # 16 seq tiles of 128

DEFAULT_OPTS = dict(reps=1, rope_add_pool=True)

_CACHED = {}


def _build_program(opts):
    reps = opts["reps"]
    nc = bacc.Bacc("TRN2", target_bir_lowering=False, debug=False,
                   num_devices=CORES)

    xT = nc.dram_tensor("xT", [D, S], BF16, kind="ExternalInput")
    wqT = nc.dram_tensor("wqT", [D, F], BF16, kind="ExternalInput")
    wkT = nc.dram_tensor("wkT", [D, F], BF16, kind="ExternalInput")
    wvT = nc.dram_tensor("wvT", [D, F], BF16, kind="ExternalInput")
    woT = nc.dram_tensor("woT", [F, D], BF16, kind="ExternalInput")
    ropeA_d = nc.dram_tensor("ropeA", [128, S], BF16, kind="ExternalInput")
    ropeB2_d = nc.dram_tensor("ropeB2", [128, S], BF16, kind="ExternalInput")
    tri_d = nc.dram_tensor("tri", [128, 128], BF16, kind="ExternalInput")

    out_d = nc.dram_tensor("partial", [S, D], BF16, kind="ExternalOutput")

    with tile.TileContext(nc) as tc, ExitStack() as ctx:
        persist = ctx.enter_context(tc.tile_pool(name="persist", bufs=1))

        # ---- persistent tiles ----
        # packed weights: wq/wk/wv are [128, 8k x 256f]; wo is [128, 2t x 1024]
        wq_t = persist.tile([128, NKT * F], BF16, tag="wq", name="wq")
        wk_t = persist.tile([128, NKT * F], BF16, tag="wk", name="wk")
        wv_t = persist.tile([128, NKT * F], BF16, tag="wv", name="wv")
        wo_t = persist.tile([128, 2 * D], BF16, tag="wo", name="wo")
        ropeA = persist.tile([128, S], BF16, tag="ropeA", name="ropeA")
        ropeB2 = persist.tile([128, S], BF16, tag="ropeB2", name="ropeB2")
        tri = persist.tile([128, 128], BF16, tag="tri", name="tri")
        # x, packed feature-major: [128, 8k x 2048s], column block sb holds
        # slices [k*2048 + 512*sb : ...] per k
        xt = persist.tile([128, NKT * S], BF16, tag="xt", name="xt")
        QT = [persist.tile([128, S], BF16, tag=f"QT{t}", name=f"QT{t}") for t in range(2)]
        KT = [persist.tile([128, S], BF16, tag=f"KT{t}", name=f"KT{t}") for t in range(2)]
        Vaug = [persist.tile([128, 260], BF16, tag=f"Vaug{st}", name=f"Vaug{st}")
                for st in range(NST)]
        attT = [persist.tile([128, S], BF16, tag=f"attT{t}", name=f"attT{t}") for t in range(2)]
        for st in range(NST):
            nc.vector.memset(Vaug[st][:, 64:260:65], 1.0)

        # ---- input loads, in consumption order ----
        def _w_load(dst, w_dram):
            # [1024, 256] dram -> [128, 8x256] sbuf, one DMA
            nc.sync.dma_start(
                out=dst.rearrange("p (k c) -> p k c", k=NKT),
                in_=w_dram.rearrange("(k p) c -> p k c", p=128))

        def _w_load_half(dst, w_dram, h):
            kk = slice(NKT // 2 * h, NKT // 2 * (h + 1))
            nc.sync.dma_start(
                out=dst.rearrange("p (k c) -> p k c", k=NKT)[:, kk],
                in_=w_dram.rearrange("(k p) c -> p k c", p=128)[:, kk])

        _w_load_half(wq_t, wqT, 0)
        _w_load_half(wq_t, wqT, 1)
        for sb in range(NSB):
            cs = slice(512 * sb, 512 * (sb + 1))
            for k in range(NKT):
                nc.sync.dma_start(
                    out=xt[:, S * k + 512 * sb: S * k + 512 * (sb + 1)],
                    in_=xT[128 * k:128 * (k + 1), cs])
            if sb == 0:
                _w_load(wk_t, wkT)
                nc.sync.dma_start(out=ropeA, in_=ropeA_d[:, :])
                nc.sync.dma_start(out=ropeB2, in_=ropeB2_d[:, :])
                _w_load(wv_t, wvT)
        nc.sync.dma_start(
            out=wo_t.rearrange("p (t c) -> p t c", t=2),
            in_=woT.rearrange("(t p) c -> p t c", p=128))
        nc.sync.dma_start(out=tri, in_=tri_d[:, :])

        env = dict(
            wq_t=wq_t, wk_t=wk_t, wv_t=wv_t, wo_t=wo_t, ropeA=ropeA,
            ropeB2=ropeB2, tri=tri, xt=xt, QT=QT, KT=KT,
            Vaug=Vaug, attT=attT, out_d=out_d,
        )
        for _rep in range(reps):
            _body(nc, tc, opts, env)

    nc.compile()
    return nc


def _body(nc, tc, opts, env):
    wq_t = env["wq_t"]; wk_t = env["wk_t"]; wv_t = env["wv_t"]
    wo_t = env["wo_t"]; ropeA = env["ropeA"]; ropeB2 = env["ropeB2"]
    tri = env["tri"]; xt = env["xt"]
    QT = env["QT"]; KT = env["KT"]; Vaug = env["Vaug"]; attT = env["attT"]
    out_d = env["out_d"]

    def xts(k, s0, s1):
        return xt[:, S * k + s0: S * k + s1]

    with tc.tile_pool(name="ptmp", bufs=4) as ptmp, \
         tc.tile_pool(name="epool", bufs=8) as epool, \
         tc.tile_pool(name="ntmp", bufs=6) as ntmp, \
         tc.tile_pool(name="opool", bufs=6) as opool:

        def attention_block(j, score_alloc, pn_alloc, wo_queue, wo_spread,
                            emit_wo_tile, half_tiles=False, passes=None):
            """Causal attention for query block j (4 per-head passes).

            half_tiles: allocate per-key-tile [128,512] score tiles and exp
            each half separately (used by the phase-A minis: same PSUM
            footprint buys a deeper ring at the cost of extra ACT overhead,
            which phase A has slack for)."""
            qs = slice(512 * j, 512 * (j + 1))
            n_pair = 2 * (j + 1)
            for (hp, hh) in (passes if passes is not None
                             else [(0, 0), (0, 1), (1, 0), (1, 1)]):
                    t = hp
                    rs = slice(64 * hh, 64 * (hh + 1))
                    h = 2 * hp + hh
                    vc = slice(65 * (h % HPC), 65 * (h % HPC) + 65)
                    pn = pn_alloc()

                    def emit_av(p, Epair, roffs):
                        ra, rb = roffs
                        nc.tensor.matmul(pn[:, ra:512],
                                         Vaug[2 * p][:, vc],
                                         Epair[0][:, ra:512],
                                         start=(p == 0), stop=False)
                        nc.tensor.matmul(pn[:, rb:512],
                                         Vaug[2 * p + 1][:, vc],
                                         Epair[1][:, rb:512],
                                         start=False, stop=(p == n_pair - 1))

                    pend = None
                    for p in range(n_pair):
                        diag = p >= n_pair - 2
                        r0 = 256 * (p - (n_pair - 2)) if diag else 0
                        roffs = (r0, r0 + 128) if diag else (0, 0)
                        if half_tiles:
                            Epair = []
                            for (sk, r) in ((2 * p, roffs[0]),
                                            (2 * p + 1, roffs[1])):
                                ks = slice(128 * sk, 128 * (sk + 1))
                                qsr = slice(512 * j + r, 512 * (j + 1))
                                psH = score_alloc()
                                nc.tensor.matmul(psH[:, r:512],
                                                 KT[t][rs, ks], QT[t][rs, qsr],
                                                 start=True, stop=True)
                                Eh = epool.tile([128, 512], BF16, tag="Eh",
                                                name="Eh")
                                nc.scalar.activation(out=Eh[:, r:512],
                                                     in_=psH[:, r:512],
                                                     func=EXP, scale=SCALE)
                                if diag:
                                    nc.vector.tensor_mul(
                                        Eh[:, r:r + 128],
                                        Eh[:, r:r + 128], tri)
                                Epair.append(Eh)
                        else:
                            psS = score_alloc()
                            for (sk, hbase, r) in ((2 * p, 0, roffs[0]),
                                                   (2 * p + 1, 512, roffs[1])):
                                ks = slice(128 * sk, 128 * (sk + 1))
                                qsr = slice(512 * j + r, 512 * (j + 1))
                                nc.tensor.matmul(psS[:, hbase + r:hbase + 512],
                                                 KT[t][rs, ks], QT[t][rs, qsr],
                                                 start=True, stop=True)
                            E = epool.tile([128, 1024], BF16, tag="E", name="E")
                            if not diag:
                                nc.scalar.activation(out=E, in_=psS,
                                                     func=EXP, scale=SCALE)
                            else:
                                for (hbase, r) in ((0, roffs[0]),
                                                   (512, roffs[1])):
                                    nc.scalar.activation(
                                        out=E[:, hbase + r:hbase + 512],
                                        in_=psS[:, hbase + r:hbase + 512],
                                        func=EXP, scale=SCALE)
                                    nc.vector.tensor_mul(
                                        E[:, hbase + r:hbase + r + 128],
                                        E[:, hbase + r:hbase + r + 128], tri)
                            Epair = (E[:, 0:512], E[:, 512:1024])
                        if wo_queue and p == min(1, n_pair - 1):
                            for _ in range(min(wo_spread, len(wo_queue))):
                                emit_wo_tile(*wo_queue.pop(0))
                        if pend is not None:
                            emit_av(*pend)
                        pend = (p, Epair, roffs)
                    emit_av(*pend)
                    # normalize -> attT
                    rc1 = ntmp.tile([1, 512], F32, tag="rc1", name="rc1")
                    nc.vector.reciprocal(rc1, pn[64:65, :])
                    rcb = ntmp.tile([64, 512], F32, tag="rcb", name="rcb")
                    nc.gpsimd.partition_broadcast(rcb, rc1, channels=64)
                    if not (j == NSB - 1 and hp == 1 and hh == 1):
                        nc.vector.tensor_mul(attT[t][rs, qs], pn[0:64, :], rcb)
                    else:
                        # final pass: normalize per 128-col chunk and emit the
                        # last block's Wo tiles progressively (shrinks tail)
                        for st in range(4):
                            c = slice(128 * st, 128 * (st + 1))
                            qc = slice(512 * j + 128 * st,
                                       512 * j + 128 * (st + 1))
                            nc.vector.tensor_mul(attT[t][rs, qc],
                                                 pn[0:64, c], rcb[:, c])
                            emit_wo_tile(j, st, 0, ob_act=False)
                            emit_wo_tile(j, st, 1, ob_act=True)

        # ---- Phase A: QKV projections + RoPE; attention j=0,1 overlapped ----
        with tc.tile_pool(name="psProj", bufs=3, space="PSUM") as psProj, \
             tc.tile_pool(name="psV", bufs=2, space="PSUM") as psV, \
             tc.tile_pool(name="psM", bufs=1, space="PSUM") as psM:
            for sb in range(NSB):
                c0 = 512 * sb
                for (w_t, dest) in ((wq_t, QT), (wk_t, KT)):
                    for t in range(2):
                        ps = psProj.tile([128, 512], F32, tag="proj", name="proj")
                        for k in range(NKT):
                            lhsT = w_t[:, F * k + 128 * t: F * k + 128 * (t + 1)]
                            nc.tensor.matmul(ps, lhsT, xts(k, c0, c0 + 512),
                                             start=(k == 0), stop=(k == NKT - 1))
                        # rope: dest = raw*A + swap16(raw)*B2; the host
                        # interleaves even/odd pairs at 16-row granularity so
                        # the swap stays within DVE 32-row quadrants
                        raw = ptmp.tile([128, 512], BF16, tag="raw", name="raw")
                        nc.scalar.copy(raw, ps)
                        rsw = ptmp.tile([128, 512], BF16, tag="rsw", name="rsw")
                        nc.vector.stream_shuffle(
                            rsw, raw, [(i + 16) % 32 for i in range(32)])
                        t1 = ptmp.tile([128, 512], BF16, tag="t1", name="t1")
                        nc.vector.tensor_mul(t1, raw, ropeA[:, c0:c0 + 512])
                        t2 = ptmp.tile([128, 512], BF16, tag="t2", name="t2")
                        nc.vector.tensor_mul(t2, rsw, ropeB2[:, c0:c0 + 512])
                        if opts["rope_add_pool"]:
                            nc.gpsimd.tensor_add(dest[t][:, c0:c0 + 512], t1, t2)
                        else:
                            nc.vector.tensor_add(dest[t][:, c0:c0 + 512], t1, t2)
                # V projection for this block (seq-major)
                for st in range(4 * sb, 4 * sb + 4):
                    s0 = 128 * st
                    ps = psV.tile([128, 256], F32, tag="projv", name="projv")
                    for k in range(NKT):
                        nc.tensor.matmul(ps, xts(k, s0, s0 + 128),
                                         wv_t[:, F * k: F * (k + 1)],
                                         start=(k == 0), stop=(k == NKT - 1))
                    dst = Vaug[st][:, 0:260].rearrange("p (h c) -> p h c", h=HPC)
                    nc.scalar.copy(dst[:, :, 0:64],
                                   ps.rearrange("p (h c) -> p h c", h=HPC))
                # early attention for blocks 0 and 1 overlaps the remaining
                # projections (mini psum rings; Wo deferred to phase B)
                if sb <= 1:
                    attention_block(
                        sb,
                        lambda: psM.tile([128, 512], F32, tag="scoreM",
                                         name="scoreM", bufs=2),
                        lambda: psM.tile([65, 512], F32, tag="pnM",
                                         name="pnM", bufs=1),
                        [], 0, None, half_tiles=True)

        # ---- Phase B: attention j=2,3 + all Wo tiles ----
        with tc.tile_pool(name="psA", bufs=2, space="PSUM") as psA, \
             tc.tile_pool(name="psB", bufs=2, space="PSUM") as psB:
            def emit_wo_tile(j, st, ot, ob_act=False):
                stg = 4 * j + st
                ss = slice(128 * stg, 128 * (stg + 1))
                os_ = slice(512 * ot, 512 * (ot + 1))
                pw = psB.tile([128, 512], F32, tag="pw", name="pw")
                for t in range(2):
                    nc.tensor.matmul(pw, attT[t][:, ss],
                                     wo_t[:, D * t + 512 * ot: D * t + 512 * (ot + 1)],
                                     start=(t == 0), stop=(t == 1))
                ob = opool.tile([128, 512], BF16, tag="ob", name="ob")
                if ob_act:
                    nc.scalar.copy(ob, pw)
                else:
                    nc.vector.tensor_copy(ob, pw)
                nc.sync.dma_start(out=out_d[ss, os_], in_=ob)

            wo_tiles = lambda j: [(j, s, o) for s in range(4) for o in range(2)]
            attention_block(
                2,
                lambda: psA.tile([128, 1024], F32, tag="score", name="score"),
                lambda: psB.tile([65, 512], F32, tag="pn", name="pn"),
                wo_tiles(0) + wo_tiles(1), 4, emit_wo_tile)
            attention_block(
                3,
                lambda: psA.tile([128, 1024], F32, tag="score", name="score"),
                lambda: psB.tile([65, 512], F32, tag="pn", name="pn"),
                wo_tiles(2), 2, emit_wo_tile)


def get_program(use_rs=False, reps=1, **kw):
    opts = dict(DEFAULT_OPTS)
    opts.update(reps=reps, **kw)
    key = tuple(sorted(opts.items()))
    if key not in _CACHED:
        _CACHED[key] = _build_program(opts)
    return _CACHED[key]


def make_in_maps(x, Wq, Wk, Wv, Wo, token_positions):
    """Host-side sharding: per-core input dicts."""
    import ml_dtypes
    bf16 = ml_dtypes.bfloat16
    x = np.asarray(x, dtype=np.float32)
    Wq = np.asarray(Wq, dtype=np.float32)
    Wk = np.asarray(Wk, dtype=np.float32)
    Wv = np.asarray(Wv, dtype=np.float32)
    Wo = np.asarray(Wo, dtype=np.float32)
    pos = np.asarray(token_positions).astype(np.float32)

    # rope tables, feature-major [128, S]. Row layout per 64-row head
    # block (q = r//32 quadrant, m = r%16): rows [32q+0:32q+16) hold the
    # even (x1) components of pairs 16q+m, rows [32q+16:32q+32) the odd
    # (x2) components -- so the rope swap partner is 16 rows away, within
    # one DVE 32-row quadrant (stream_shuffle-able).
    i = np.arange(DH // 2, dtype=np.float32)
    d = THETA ** (2.0 * i / DH)                       # [32]
    tt = pos[None, :] / d[:, None]                    # [32, S]
    sin, cos = np.sin(tt), np.cos(tt)
    r64 = np.arange(64)
    p_idx = 16 * (r64 // 32) + (r64 % 16)             # pair index per row
    half = (r64 % 32) // 16                           # 0 = even, 1 = odd
    A64 = cos[p_idx, :]
    B64 = np.where(half[:, None] == 0, -sin[p_idx, :], sin[p_idx, :])
    A = np.tile(A64, (2, 1)).astype(bf16)             # [128, S]
    B2 = np.tile(B64, (2, 1)).astype(bf16)

    # causal triangle mask [128, 128]: allow j >= p
    p = np.arange(128)[:, None]
    jj = np.arange(128)[None, :]
    tri = (jj >= p).astype(bf16)

    # per-head Q/K row permutation matching the interleaved layout
    rows64 = np.array([2 * (16 * q + m) + hf
                       for q in range(2) for hf in range(2)
                       for m in range(16)])
    perm_rows = np.concatenate([64 * h + rows64 for h in range(H)])

    in_maps = []
    for c in range(CORES):
        b, g = c // 4, c % 4
        rows = perm_rows[F * g:F * (g + 1)]
        nat = np.arange(F * g, F * (g + 1))
        in_maps.append({
            "xT": np.ascontiguousarray(x[b].T).astype(bf16),
            "wqT": np.ascontiguousarray(Wq[rows, :].T).astype(bf16),
            "wkT": np.ascontiguousarray(Wk[rows, :].T).astype(bf16),
            "wvT": np.ascontiguousarray(Wv[nat, :].T).astype(bf16),
            "woT": np.ascontiguousarray(Wo[:, nat].T).astype(bf16),
            "ropeA": A,
            "ropeB2": B2,
            "tri": tri,
        })
    return in_maps


def kernel(x, Wq, Wk, Wv, Wo, token_positions):
    from concourse.bass_utils import run_bass_kernel_spmd
    nc = get_program(False)
    in_maps = make_in_maps(x, Wq, Wk, Wv, Wo, token_positions)
    res = run_bass_kernel_spmd(nc, in_maps, list(range(CORES)))
    out = np.empty((B, S, D), dtype=np.float32)
    for b in range(B):
        acc = res.results[4 * b]["partial"].astype(np.float32).copy()
        for g in range(1, 4):
            acc += res.results[4 * b + g]["partial"]
        out[b] = acc
    return out


# revision 77
# speedup vs baseline: 1.0228x; 1.0022x over previous
"""Multi-head self-attention (RoPE, causal) Trainium2 Bass kernel.

Sharding: 8 cores = 2 batches x 4 head-groups (4 heads each).
Each core computes QKV projections for its heads (feature-major via x^T),
RoPE, causal attention with transposed scores (softmax along partitions
handled via exp + ones-column denominator in the V matmul), and a partial
output projection over its head slice. The host sums the 4 partials per
batch (reduce step of the tensor-parallel output projection).

v2: bf16 data paths (inputs, Q/K/V, E, attT, weights, output partials;
host upcasts before the 4-partial reduce) with fp32 PSUM accumulation;
ordered input DMAs so compute starts after ~2 MB arrives;
the RoPE even/odd swap is a DVE stream_shuffle (host interleaves pair
components at 16-row granularity so swap partners share a 32-row DVE
quadrant); softmax denominator broadcast via gpsimd
partition_broadcast (no PE broadcast matmul); elementwise work spread
across ACT/DVE/Pool.

Scheduling: attention for query blocks 0-1 is emitted inside phase A on
mini PSUM rings, hiding it under the remaining QKV projections; per-head
attention passes use single-alloc score tiles so a 2-deep ring gives a
full pair of lookahead for the ACT exp stream; AV matmuls trail one pair
behind scores; Wo tiles of block j-1 are spread across block j's passes
as PE filler; the final pass normalizes per 128-col chunk and drains the
last Wo tiles progressively.

TimelineSim: 144.7 us vs 224.5 us for the fp32r baseline (measured
258.5 us on HW); HW rel err vs reference 4.8e-3.
"""
import math
from contextlib import ExitStack

import numpy as np

import concourse.tile as tile
from concourse import bacc, mybir

F32 = mybir.dt.float32
BF16 = mybir.dt.bfloat16
EXP = mybir.ActivationFunctionType.Exp

B, S, D, H, DH = 2, 2048, 1024, 16, 64
THETA = 10000.0
CORES = 8
HPC = 4                    # heads per core
F = HPC * DH               # 256 features per core
SCALE = 1.0 / math.sqrt(DH)
NKT = D // 128             # 8 k tiles
NSB = S // 512             # 4 seq blocks of 512
NST = S // 128             # 16 seq tiles of 128

DEFAULT_OPTS = dict(reps=1, rope_add_pool=True)

_CACHED = {}


def _build_program(opts):
    reps = opts["reps"]
    nc = bacc.Bacc("TRN2", target_bir_lowering=False, debug=False,
                   num_devices=CORES)

    xT = nc.dram_tensor("xT", [D, S], BF16, kind="ExternalInput")
    wqT = nc.dram_tensor("wqT", [D, F], BF16, kind="ExternalInput")
    wkT = nc.dram_tensor("wkT", [D, F], BF16, kind="ExternalInput")
    wvT = nc.dram_tensor("wvT", [D, F], BF16, kind="ExternalInput")
    woT = nc.dram_tensor("woT", [F, D], BF16, kind="ExternalInput")
    ropeA_d = nc.dram_tensor("ropeA", [128, S], BF16, kind="ExternalInput")
    ropeB2_d = nc.dram_tensor("ropeB2", [128, S], BF16, kind="ExternalInput")
    tri_d = nc.dram_tensor("tri", [128, 128], BF16, kind="ExternalInput")

    out_d = nc.dram_tensor("partial", [S, D], BF16, kind="ExternalOutput")

    with tile.TileContext(nc) as tc, ExitStack() as ctx:
        persist = ctx.enter_context(tc.tile_pool(name="persist", bufs=1))

        # ---- persistent tiles ----
        # packed weights: wq/wk/wv are [128, 8k x 256f]; wo is [128, 2t x 1024]
        wq_t = persist.tile([128, NKT * F], BF16, tag="wq", name="wq")
        wk_t = persist.tile([128, NKT * F], BF16, tag="wk", name="wk")
        wv_t = persist.tile([128, NKT * F], BF16, tag="wv", name="wv")
        wo_t = persist.tile([128, 2 * D], BF16, tag="wo", name="wo")
        ropeA = persist.tile([128, S], BF16, tag="ropeA", name="ropeA")
        ropeB2 = persist.tile([128, S], BF16, tag="ropeB2", name="ropeB2")
        tri = persist.tile([128, 128], BF16, tag="tri", name="tri")
        # x, packed feature-major: [128, 8k x 2048s], column block sb holds
        # slices [k*2048 + 512*sb : ...] per k
        xt = persist.tile([128, NKT * S], BF16, tag="xt", name="xt")
        QT = [persist.tile([128, S], BF16, tag=f"QT{t}", name=f"QT{t}") for t in range(2)]
        KT = [persist.tile([128, S], BF16, tag=f"KT{t}", name=f"KT{t}") for t in range(2)]
        Vaug = [persist.tile([128, 260], BF16, tag=f"Vaug{st}", name=f"Vaug{st}")
                for st in range(NST)]
        attT = [persist.tile([128, S], BF16, tag=f"attT{t}", name=f"attT{t}") for t in range(2)]
        for st in range(NST):
            nc.vector.memset(Vaug[st][:, 64:260:65], 1.0)

        # ---- input loads, in consumption order ----
        def _w_load(dst, w_dram):
            # [1024, 256] dram -> [128, 8x256] sbuf, one DMA
            nc.sync.dma_start(
                out=dst.rearrange("p (k c) -> p k c", k=NKT),
                in_=w_dram.rearrange("(k p) c -> p k c", p=128))

        def _w_load_half(dst, w_dram, h):
            kk = slice(NKT // 2 * h, NKT // 2 * (h + 1))
            nc.sync.dma_start(
                out=dst.rearrange("p (k c) -> p k c", k=NKT)[:, kk],
                in_=w_dram.rearrange("(k p) c -> p k c", p=128)[:, kk])

        _w_load_half(wq_t, wqT, 0)
        _w_load_half(wq_t, wqT, 1)
        for sb in range(NSB):
            cs = slice(512 * sb, 512 * (sb + 1))
            for k in range(NKT):
                nc.sync.dma_start(
                    out=xt[:, S * k + 512 * sb: S * k + 512 * (sb + 1)],
                    in_=xT[128 * k:128 * (k + 1), cs])
            if sb == 0:
                _w_load(wk_t, wkT)
                nc.sync.dma_start(out=ropeA, in_=ropeA_d[:, :])
                nc.sync.dma_start(out=ropeB2, in_=ropeB2_d[:, :])
                _w_load(wv_t, wvT)
        nc.sync.dma_start(
            out=wo_t.rearrange("p (t c) -> p t c", t=2),
            in_=woT.rearrange("(t p) c -> p t c", p=128))
        nc.sync.dma_start(out=tri, in_=tri_d[:, :])

        env = dict(
            wq_t=wq_t, wk_t=wk_t, wv_t=wv_t, wo_t=wo_t, ropeA=ropeA,
            ropeB2=ropeB2, tri=tri, xt=xt, QT=QT, KT=KT,
            Vaug=Vaug, attT=attT, out_d=out_d,
        )
        for _rep in range(reps):
            _body(nc, tc, opts, env)

    nc.compile()
    return nc


def _body(nc, tc, opts, env):
    wq_t = env["wq_t"]; wk_t = env["wk_t"]; wv_t = env["wv_t"]
    wo_t = env["wo_t"]; ropeA = env["ropeA"]; ropeB2 = env["ropeB2"]
    tri = env["tri"]; xt = env["xt"]
    QT = env["QT"]; KT = env["KT"]; Vaug = env["Vaug"]; attT = env["attT"]
    out_d = env["out_d"]

    def xts(k, s0, s1):
        return xt[:, S * k + s0: S * k + s1]

    with tc.tile_pool(name="ptmp", bufs=4) as ptmp, \
         tc.tile_pool(name="epool", bufs=8) as epool, \
         tc.tile_pool(name="ntmp", bufs=6) as ntmp, \
         tc.tile_pool(name="opool", bufs=6) as opool:

        def attention_block(j, score_alloc, pn_alloc, wo_queue, wo_spread,
                            emit_wo_tile, half_tiles=False, passes=None):
            """Causal attention for query block j (4 per-head passes).

            half_tiles: allocate per-key-tile [128,512] score tiles and exp
            each half separately (used by the phase-A minis: same PSUM
            footprint buys a deeper ring at the cost of extra ACT overhead,
            which phase A has slack for)."""
            qs = slice(512 * j, 512 * (j + 1))
            n_pair = 2 * (j + 1)
            for (hp, hh) in (passes if passes is not None
                             else [(0, 0), (0, 1), (1, 0), (1, 1)]):
                    t = hp
                    rs = slice(64 * hh, 64 * (hh + 1))
                    h = 2 * hp + hh
                    vc = slice(65 * (h % HPC), 65 * (h % HPC) + 65)
                    pn = pn_alloc()

                    def emit_av(p, Epair, roffs):
                        ra, rb = roffs
                        nc.tensor.matmul(pn[:, ra:512],
                                         Vaug[2 * p][:, vc],
                                         Epair[0][:, ra:512],
                                         start=(p == 0), stop=False)
                        nc.tensor.matmul(pn[:, rb:512],
                                         Vaug[2 * p + 1][:, vc],
                                         Epair[1][:, rb:512],
                                         start=False, stop=(p == n_pair - 1))

                    pend = None
                    for p in range(n_pair):
                        diag = p >= n_pair - 2
                        r0 = 256 * (p - (n_pair - 2)) if diag else 0
                        roffs = (r0, r0 + 128) if diag else (0, 0)
                        if half_tiles:
                            Epair = []
                            for (sk, r) in ((2 * p, roffs[0]),
                                            (2 * p + 1, roffs[1])):
                                ks = slice(128 * sk, 128 * (sk + 1))
                                qsr = slice(512 * j + r, 512 * (j + 1))
                                psH = score_alloc()
                                nc.tensor.matmul(psH[:, r:512],
                                                 KT[t][rs, ks], QT[t][rs, qsr],
                                                 start=True, stop=True)
                                Eh = epool.tile([128, 512], BF16, tag="Eh",
                                                name="Eh")
                                nc.scalar.activation(out=Eh[:, r:512],
                                                     in_=psH[:, r:512],
                                                     func=EXP, scale=SCALE)
                                if diag:
                                    nc.vector.tensor_mul(
                                        Eh[:, r:r + 128],
                                        Eh[:, r:r + 128], tri)
                                Epair.append(Eh)
                        else:
                            psS = score_alloc()
                            for (sk, hbase, r) in ((2 * p, 0, roffs[0]),
                                                   (2 * p + 1, 512, roffs[1])):
                                ks = slice(128 * sk, 128 * (sk + 1))
                                qsr = slice(512 * j + r, 512 * (j + 1))
                                nc.tensor.matmul(psS[:, hbase + r:hbase + 512],
                                                 KT[t][rs, ks], QT[t][rs, qsr],
                                                 start=True, stop=True)
                            E = epool.tile([128, 1024], BF16, tag="E", name="E")
                            if not diag:
                                nc.scalar.activation(out=E, in_=psS,
                                                     func=EXP, scale=SCALE)
                            else:
                                for (hbase, r) in ((0, roffs[0]),
                                                   (512, roffs[1])):
                                    nc.scalar.activation(
                                        out=E[:, hbase + r:hbase + 512],
                                        in_=psS[:, hbase + r:hbase + 512],
                                        func=EXP, scale=SCALE)
                                    nc.vector.tensor_mul(
                                        E[:, hbase + r:hbase + r + 128],
                                        E[:, hbase + r:hbase + r + 128], tri)
                            Epair = (E[:, 0:512], E[:, 512:1024])
                        if wo_queue and p == min(1, n_pair - 1):
                            for _ in range(min(wo_spread, len(wo_queue))):
                                emit_wo_tile(*wo_queue.pop(0))
                        if pend is not None:
                            emit_av(*pend)
                        pend = (p, Epair, roffs)
                    emit_av(*pend)
                    # normalize -> attT
                    rc1 = ntmp.tile([1, 512], F32, tag="rc1", name="rc1")
                    nc.vector.reciprocal(rc1, pn[64:65, :])
                    rcb = ntmp.tile([64, 512], F32, tag="rcb", name="rcb")
                    nc.gpsimd.partition_broadcast(rcb, rc1, channels=64)
                    if not (j == NSB - 1 and hp == 1 and hh == 1):
                        nc.vector.tensor_mul(attT[t][rs, qs], pn[0:64, :], rcb)
                    else:
                        # final pass: normalize per 128-col chunk and emit the
                        # last block's Wo tiles progressively (shrinks tail)
                        for st in range(4):
                            c = slice(128 * st, 128 * (st + 1))
                            qc = slice(512 * j + 128 * st,
                                       512 * j + 128 * (st + 1))
                            nc.vector.tensor_mul(attT[t][rs, qc],
                                                 pn[0:64, c], rcb[:, c])
                            emit_wo_tile(j, st, 0, ob_act=False)
                            emit_wo_tile(j, st, 1, ob_act=True)

        # ---- Phase A: QKV projections + RoPE; attention j=0,1 overlapped ----
        with tc.tile_pool(name="psProj", bufs=2, space="PSUM") as psProj, \
             tc.tile_pool(name="psV", bufs=2, space="PSUM") as psV, \
             tc.tile_pool(name="psM", bufs=1, space="PSUM") as psM:
            for sb in range(NSB):
                c0 = 512 * sb
                for (w_t, dest) in ((wq_t, QT), (wk_t, KT)):
                    for t in range(2):
                        ps = psProj.tile([128, 512], F32, tag="proj", name="proj")
                        for k in range(NKT):
                            lhsT = w_t[:, F * k + 128 * t: F * k + 128 * (t + 1)]
                            nc.tensor.matmul(ps, lhsT, xts(k, c0, c0 + 512),
                                             start=(k == 0), stop=(k == NKT - 1))
                        # rope: dest = raw*A + swap16(raw)*B2; the host
                        # interleaves even/odd pairs at 16-row granularity so
                        # the swap stays within DVE 32-row quadrants
                        raw = ptmp.tile([128, 512], BF16, tag="raw", name="raw")
                        nc.scalar.copy(raw, ps)
                        rsw = ptmp.tile([128, 512], BF16, tag="rsw", name="rsw")
                        nc.vector.stream_shuffle(
                            rsw, raw, [(i + 16) % 32 for i in range(32)])
                        t1 = ptmp.tile([128, 512], BF16, tag="t1", name="t1")
                        nc.vector.tensor_mul(t1, raw, ropeA[:, c0:c0 + 512])
                        t2 = ptmp.tile([128, 512], BF16, tag="t2", name="t2")
                        nc.vector.tensor_mul(t2, rsw, ropeB2[:, c0:c0 + 512])
                        if opts["rope_add_pool"]:
                            nc.gpsimd.tensor_add(dest[t][:, c0:c0 + 512], t1, t2)
                        else:
                            nc.vector.tensor_add(dest[t][:, c0:c0 + 512], t1, t2)
                # V projection for this block (seq-major)
                for st in range(4 * sb, 4 * sb + 4):
                    s0 = 128 * st
                    ps = psV.tile([128, 256], F32, tag="projv", name="projv")
                    for k in range(NKT):
                        nc.tensor.matmul(ps, xts(k, s0, s0 + 128),
                                         wv_t[:, F * k: F * (k + 1)],
                                         start=(k == 0), stop=(k == NKT - 1))
                    dst = Vaug[st][:, 0:260].rearrange("p (h c) -> p h c", h=HPC)
                    nc.scalar.copy(dst[:, :, 0:64],
                                   ps.rearrange("p (h c) -> p h c", h=HPC))
                # early attention for blocks 0 and 1 overlaps the remaining
                # projections (mini psum rings; Wo deferred to phase B)
                if sb <= 1:
                    attention_block(
                        sb,
                        lambda: psM.tile([128, 512], F32, tag="scoreM",
                                         name="scoreM", bufs=3),
                        lambda: psM.tile([65, 512], F32, tag="pnM",
                                         name="pnM", bufs=1),
                        [], 0, None, half_tiles=True)

        # ---- Phase B: attention j=2,3 + all Wo tiles ----
        with tc.tile_pool(name="psA", bufs=2, space="PSUM") as psA, \
             tc.tile_pool(name="psB", bufs=2, space="PSUM") as psB:
            def emit_wo_tile(j, st, ot, ob_act=False):
                stg = 4 * j + st
                ss = slice(128 * stg, 128 * (stg + 1))
                os_ = slice(512 * ot, 512 * (ot + 1))
                pw = psB.tile([128, 512], F32, tag="pw", name="pw")
                for t in range(2):
                    nc.tensor.matmul(pw, attT[t][:, ss],
                                     wo_t[:, D * t + 512 * ot: D * t + 512 * (ot + 1)],
                                     start=(t == 0), stop=(t == 1))
                ob = opool.tile([128, 512], BF16, tag="ob", name="ob")
                if ob_act:
                    nc.scalar.copy(ob, pw)
                else:
                    nc.vector.tensor_copy(ob, pw)
                nc.sync.dma_start(out=out_d[ss, os_], in_=ob)

            wo_tiles = lambda j: [(j, s, o) for s in range(4) for o in range(2)]
            attention_block(
                2,
                lambda: psA.tile([128, 1024], F32, tag="score", name="score"),
                lambda: psB.tile([65, 512], F32, tag="pn", name="pn"),
                wo_tiles(0) + wo_tiles(1), 4, emit_wo_tile)
            attention_block(
                3,
                lambda: psA.tile([128, 1024], F32, tag="score", name="score"),
                lambda: psB.tile([65, 512], F32, tag="pn", name="pn"),
                wo_tiles(2), 2, emit_wo_tile)


def get_program(use_rs=False, reps=1, **kw):
    opts = dict(DEFAULT_OPTS)
    opts.update(reps=reps, **kw)
    key = tuple(sorted(opts.items()))
    if key not in _CACHED:
        _CACHED[key] = _build_program(opts)
    return _CACHED[key]


def make_in_maps(x, Wq, Wk, Wv, Wo, token_positions):
    """Host-side sharding: per-core input dicts."""
    import ml_dtypes
    bf16 = ml_dtypes.bfloat16
    x = np.asarray(x, dtype=np.float32)
    Wq = np.asarray(Wq, dtype=np.float32)
    Wk = np.asarray(Wk, dtype=np.float32)
    Wv = np.asarray(Wv, dtype=np.float32)
    Wo = np.asarray(Wo, dtype=np.float32)
    pos = np.asarray(token_positions).astype(np.float32)

    # rope tables, feature-major [128, S]. Row layout per 64-row head
    # block (q = r//32 quadrant, m = r%16): rows [32q+0:32q+16) hold the
    # even (x1) components of pairs 16q+m, rows [32q+16:32q+32) the odd
    # (x2) components -- so the rope swap partner is 16 rows away, within
    # one DVE 32-row quadrant (stream_shuffle-able).
    i = np.arange(DH // 2, dtype=np.float32)
    d = THETA ** (2.0 * i / DH)                       # [32]
    tt = pos[None, :] / d[:, None]                    # [32, S]
    sin, cos = np.sin(tt), np.cos(tt)
    r64 = np.arange(64)
    p_idx = 16 * (r64 // 32) + (r64 % 16)             # pair index per row
    half = (r64 % 32) // 16                           # 0 = even, 1 = odd
    A64 = cos[p_idx, :]
    B64 = np.where(half[:, None] == 0, -sin[p_idx, :], sin[p_idx, :])
    A = np.tile(A64, (2, 1)).astype(bf16)             # [128, S]
    B2 = np.tile(B64, (2, 1)).astype(bf16)

    # causal triangle mask [128, 128]: allow j >= p
    p = np.arange(128)[:, None]
    jj = np.arange(128)[None, :]
    tri = (jj >= p).astype(bf16)

    # per-head Q/K row permutation matching the interleaved layout
    rows64 = np.array([2 * (16 * q + m) + hf
                       for q in range(2) for hf in range(2)
                       for m in range(16)])
    perm_rows = np.concatenate([64 * h + rows64 for h in range(H)])

    in_maps = []
    for c in range(CORES):
        b, g = c // 4, c % 4
        rows = perm_rows[F * g:F * (g + 1)]
        nat = np.arange(F * g, F * (g + 1))
        in_maps.append({
            "xT": np.ascontiguousarray(x[b].T).astype(bf16),
            "wqT": np.ascontiguousarray(Wq[rows, :].T).astype(bf16),
            "wkT": np.ascontiguousarray(Wk[rows, :].T).astype(bf16),
            "wvT": np.ascontiguousarray(Wv[nat, :].T).astype(bf16),
            "woT": np.ascontiguousarray(Wo[:, nat].T).astype(bf16),
            "ropeA": A,
            "ropeB2": B2,
            "tri": tri,
        })
    return in_maps


def kernel(x, Wq, Wk, Wv, Wo, token_positions):
    from concourse.bass_utils import run_bass_kernel_spmd
    nc = get_program(False)
    in_maps = make_in_maps(x, Wq, Wk, Wv, Wo, token_positions)
    res = run_bass_kernel_spmd(nc, in_maps, list(range(CORES)))
    out = np.empty((B, S, D), dtype=np.float32)
    for b in range(B):
        acc = res.results[4 * b]["partial"].astype(np.float32).copy()
        for g in range(1, 4):
            acc += res.results[4 * b + g]["partial"]
        out[b] = acc
    return out
